# revision 3
# baseline (speedup 1.0000x reference)
"""Self-contained Trainium2 Bass kernel for nn_ComplementarityScoreHead.

out = (h_norm @ h_norm.T) * edge_mask, h = MLP(x), h_norm = h / ||h||_2(rows)

Strategy (8 NeuronCores, SPMD, symmetric-staircase sparse readout):
  - The masked output has only ~262k nonzeros (0.4%), and corr is
    symmetric.  Each core m receives xT rolled by -1024*m node-columns,
    computes raw (unnormalized) hT = MLP(x).T for local node columns
    [0, 5120), and the raw correlation *staircase*: for each 128-row
    chunk mt, the [128, 4224] slab D[p, j] = h_{128mt+p} . h_{128mt+p+j'}
    covering symmetric offsets d in [0, 4096].  Every unordered node pair
    appears in exactly one core's staircase (offset <= 4096 side).
  - No on-device masking/normalization: the chunk diagonal D[p, p] holds
    |h|^2, so the host derives rsqrt norms from the staircase itself,
    scales the ~262k gathered edge values, and scatters them into the
    zero matrix.  Device writes bf16 staircase (8.25 MiB/core) instead of
    a dense masked 32 MiB f32 slab.
  - MLP: L1 (relu+bias on Act), L2 (bias via DVE tensor_scalar eviction)
    all fp32r matmuls on 512-col tiles; only 10 of 16 tiles are needed.
  - Corr chunk evictions (psum f32 -> sbuf bf16) round-robin across
    Act/DVE/Pool; chunk DMAs alternate 2 queues.
"""
import sys
import numpy as np

sys.path.insert(0, '/opt/trn_rl_repo')

import concourse.bass as bass  # noqa: E402
import concourse.mybir as mybir  # noqa: E402
from concourse import bacc  # noqa: E402
from concourse.tile import TileContext  # noqa: E402
from concourse.bass_utils import run_bass_kernel_spmd  # noqa: E402

N = 8192
F = 128
H = 256
NCORES = 8
SLAB = N // NCORES
CHUNKS = SLAB // 128       # 8 chunks of 128 rows per core
DMAX = 4096                # symmetric staircase max offset
CW = 128 + DMAX            # 4224 chunk window width
COLS = 128 * (CHUNKS - 1) + CW  # 5120 local node columns needed
NT = COLS // 512           # 10 MLP column tiles


def _build_nc():
    f32 = mybir.dt.float32
    f32r = mybir.dt.float32r
    bf16 = mybir.dt.bfloat16

    nc = bacc.Bacc()
    xT = nc.declare_dram_parameter("xT", [F, COLS], f32r, isOutput=False)
    W1 = nc.declare_dram_parameter("W1", [F, H], f32r, isOutput=False)
    b1 = nc.declare_dram_parameter("b1", [128, 2], f32, isOutput=False)
    W2 = nc.declare_dram_parameter("W2", [128, 2, H], f32r, isOutput=False)
    b2 = nc.declare_dram_parameter("b2", [128, 2], f32, isOutput=False)
    out = nc.declare_dram_parameter("out", [CHUNKS, 128, CW], bf16,
                                    isOutput=True)

    with TileContext(nc) as tc:
        with (
            tc.tile_pool(name="singles", bufs=1) as singles,
            tc.tile_pool(name="hn", bufs=1) as hn_pool,
            tc.tile_pool(name="xa", bufs=4) as xa_pool,
            tc.tile_pool(name="mid", bufs=3) as mid,
            tc.tile_pool(name="chunkh", bufs=3) as chunk_pool,
            tc.tile_pool(name="psA", bufs=4, space="PSUM") as psA,
            tc.tile_pool(name="psW", bufs=4, space="PSUM") as psW,
        ):
            w1r = singles.tile([128, H], f32r)
            nc.sync.dma_start(out=w1r[:], in_=W1[:])
            w2r = singles.tile([128, 2, H], f32r)
            nc.sync.dma_start(out=w2r[:], in_=W2[:])
            b1s = singles.tile([128, 2], f32)
            nc.scalar.dma_start(out=b1s[:], in_=b1[:])
            b2s = singles.tile([128, 2], f32)
            nc.scalar.dma_start(out=b2s[:], in_=b2[:])

            hnT = [hn_pool.tile([128, COLS], f32r, name=f"hnT{s}")
                   for s in range(2)]

            def mlp_tile(nt):
                sl = slice(nt * 512, (nt + 1) * 512)
                xts = xa_pool.tile([128, 512], f32r, tag="xts")
                dmae = nc.sync if nt % 2 == 0 else nc.scalar
                dmae.dma_start(out=xts[:], in_=xT[:, sl])
                r1s = mid.tile([128, 2, 512], f32r, tag="r1s")
                for s in range(2):
                    ps = psA.tile([128, 512], f32, tag="ps")
                    nc.tensor.matmul(
                        ps[:], w1r[:, s * 128:(s + 1) * 128], xts[:],
                        start=True, stop=True)
                    nc.scalar.activation(
                        r1s[:, s, :], ps[:],
                        mybir.ActivationFunctionType.Relu,
                        bias=b1s[:, s:s + 1])
                for s2 in range(2):
                    ps = psA.tile([128, 512], f32, tag="ps")
                    for k in range(2):
                        nc.tensor.matmul(
                            ps[:], w2r[:, k, s2 * 128:(s2 + 1) * 128],
                            r1s[:, k, :], start=(k == 0), stop=(k == 1))
                    nc.vector.tensor_scalar(
                        hnT[s2][:, sl], ps[:], b2s[:, s2:s2 + 1], None,
                        op0=mybir.AluOpType.add)

            def corr_chunk(mt):
                base = mt * 128
                ch = chunk_pool.tile([128, CW], bf16, tag="ch")
                for t in range(9):
                    w = 512 if t < 8 else 128
                    cs = base + t * 512
                    ps = psW.tile([128, 512], f32, tag="psw")
                    for k in range(2):
                        nc.tensor.matmul(
                            ps[:, :w], hnT[k][:, base:base + 128],
                            hnT[k][:, cs:cs + w],
                            start=(k == 0), stop=(k == 1))
                    dst = ch[:, t * 512:t * 512 + w]
                    if t % 2 == 0:
                        nc.scalar.activation(
                            dst, ps[:, :w],
                            mybir.ActivationFunctionType.Identity)
                    else:
                        nc.vector.tensor_copy(dst, ps[:, :w])
                dmao = nc.sync if mt % 2 == 0 else nc.scalar
                dmao.dma_start(out=out[mt], in_=ch[:])

            for nt in range(9):
                mlp_tile(nt)
            corr_chunk(0)
            mlp_tile(9)
            for mt in range(1, CHUNKS):
                corr_chunk(mt)
    nc.compile()
    return nc


_NC_CACHE = {}


def _get_nc():
    if "nc" not in _NC_CACHE:
        _NC_CACHE["nc"] = _build_nc()
    return _NC_CACHE["nc"]


def _prep_in_maps(x, W1, b1, W2, b2):
    x = np.asarray(x, dtype=np.float32)
    W1 = np.ascontiguousarray(np.asarray(W1, dtype=np.float32))
    W2h = np.ascontiguousarray(
        np.asarray(W2, dtype=np.float32).reshape(2, 128, H).transpose(1, 0, 2))
    b1h = np.ascontiguousarray(np.asarray(b1, dtype=np.float32).reshape(2, 128).T)
    b2h = np.ascontiguousarray(np.asarray(b2, dtype=np.float32).reshape(2, 128).T)
    in_maps = []
    for m in range(NCORES):
        xTm = np.ascontiguousarray(
            np.roll(x, -SLAB * m, axis=0).T[:, :COLS])
        in_maps.append({"xT": xTm, "W1": W1, "b1": b1h, "W2": W2h,
                        "b2": b2h})
    return in_maps


def _assemble(results, edge_index):
    buf = np.stack([np.asarray(results[m]["out"]).astype(np.float32)
                    for m in range(NCORES)])  # [8, CHUNKS, 128, CW]

    g = np.arange(N)
    diag = buf[g // SLAB, (g % SLAB) // 128, g % 128, g % 128]
    rsq = 1.0 / np.maximum(np.sqrt(np.maximum(diag, 0.0)), 1e-12)

    r = np.asarray(edge_index[0], dtype=np.int64)
    c = np.asarray(edge_index[1], dtype=np.int64)
    d = (c - r) % N
    sel = d <= DMAX
    own = np.where(sel, r, c)
    off = np.where(sel, d, N - d)
    lr = own % SLAB
    p = lr % 128
    vals = buf[own // SLAB, lr // 128, p, p + off]
    vals = vals * rsq[r] * rsq[c]

    out = np.zeros((N, N), dtype=np.float32)
    out[r, c] = vals
    return out


def kernel(x, edge_index, W1, b1, W2, b2):
    nc = _get_nc()
    in_maps = _prep_in_maps(x, W1, b1, W2, b2)
    res = run_bass_kernel_spmd(nc, in_maps, list(range(NCORES)))
    return _assemble(res.results, edge_index)


# revision 8
# speedup vs baseline: 3.0503x; 3.0503x over previous
"""Self-contained Trainium2 Bass kernel for nn_ComplementarityScoreHead.

out = (h_norm @ h_norm.T) * edge_mask, h = MLP(x), h_norm = h / ||h||_2(rows)

Strategy (8 NeuronCores, SPMD, symmetric-staircase sparse readout):
  - The masked output has only ~262k nonzeros (0.4%), and corr is
    symmetric.  Each core m receives xT rolled by -1024*m node-columns,
    computes raw (unnormalized) hT = MLP(x).T for local node columns
    [0, 5120), and the raw correlation *staircase*: for each 128-row
    chunk mt, the [128, 4224] slab D[p, j] = h_{128mt+p} . h_{128mt+j}
    covering symmetric offsets d in [0, 4096].  Every unordered node pair
    appears in at least one core's staircase (offset <= 4096 side).
  - No on-device masking/normalization: the chunk diagonal D[p, p] holds
    |h|^2, so the host derives rsqrt norms from the staircase itself,
    scales the ~262k gathered edge values, and scatters them into the
    zero matrix.  Device writes bf16 staircase (8.25 MiB/core) instead of
    a dense masked 32 MiB f32 slab.
  - MLP: L1 (relu+bias on Act), L2 (bias via DVE tensor_scalar eviction),
    all fp32r matmuls on 512-col tiles; only 10 of 16 tiles are needed.
  - Corr: 1024-wide psum tiles, evictions (psum f32 -> sbuf bf16)
    alternate Act/DVE; chunk DMAs split in half across SP + Pool queues
    so no compute engine is charged for DMA time.
"""
import sys
import numpy as np

sys.path.insert(0, '/opt/trn_rl_repo')

import concourse.bass as bass  # noqa: E402
import concourse.mybir as mybir  # noqa: E402
from concourse import bacc  # noqa: E402
from concourse.tile import TileContext  # noqa: E402
from concourse.bass_utils import run_bass_kernel_spmd  # noqa: E402

N = 8192
F = 128
H = 256
NCORES = 8
SLAB = N // NCORES
CHUNKS = SLAB // 128       # 8 chunks of 128 rows per core
DMAX = 4096                # symmetric staircase max offset
CW = 128 + DMAX            # 4224 chunk window width
COLS = 128 * (CHUNKS - 1) + CW  # 5120 local node columns needed
NT = COLS // 512           # 10 MLP column tiles


def _build_nc():
    f32 = mybir.dt.float32
    f32r = mybir.dt.float32r
    bf16 = mybir.dt.bfloat16

    nc = bacc.Bacc()
    xT = nc.declare_dram_parameter("xT", [F, COLS], f32r, isOutput=False)
    W1 = nc.declare_dram_parameter("W1", [F, H], f32r, isOutput=False)
    b1 = nc.declare_dram_parameter("b1", [128, 2], f32, isOutput=False)
    W2 = nc.declare_dram_parameter("W2", [128, 2, H], f32r, isOutput=False)
    b2 = nc.declare_dram_parameter("b2", [128, 2], f32, isOutput=False)
    out = nc.declare_dram_parameter("out", [CHUNKS, 128, CW], bf16,
                                    isOutput=True)

    with TileContext(nc) as tc:
        with (
            tc.tile_pool(name="singles", bufs=1) as singles,
            tc.tile_pool(name="hn", bufs=1) as hn_pool,
            tc.tile_pool(name="xa", bufs=4) as xa_pool,
            tc.tile_pool(name="mid", bufs=3) as mid,
            tc.tile_pool(name="chunkh", bufs=3) as chunk_pool,
            tc.tile_pool(name="psA", bufs=4, space="PSUM") as psA,
            tc.tile_pool(name="psW", bufs=4, space="PSUM") as psW,
        ):
            # Prefetch first x tile on the (otherwise idle at t=0) Pool
            # queue so the PE can start as early as possible.
            xts0 = xa_pool.tile([128, 512], f32r, tag="xts")
            nc.gpsimd.dma_start(out=xts0[:], in_=xT[:, 0:512])
            w1r = singles.tile([128, H], f32r)
            nc.sync.dma_start(out=w1r[:], in_=W1[:])
            w2r = singles.tile([128, 2, H], f32r)
            nc.sync.dma_start(out=w2r[:], in_=W2[:])
            b1s = singles.tile([128, 2], f32)
            nc.scalar.dma_start(out=b1s[:], in_=b1[:])
            b2s = singles.tile([128, 2], f32)
            nc.scalar.dma_start(out=b2s[:], in_=b2[:])

            hnT = [hn_pool.tile([128, COLS], f32r, name=f"hnT{s}")
                   for s in range(2)]

            def mlp_tile(nt):
                sl = slice(nt * 512, (nt + 1) * 512)
                if nt == 0:
                    xts = xts0
                else:
                    xts = xa_pool.tile([128, 512], f32r, tag="xts")
                    dmae = nc.sync if nt % 2 == 0 else nc.gpsimd
                    dmae.dma_start(out=xts[:], in_=xT[:, sl])
                r1s = mid.tile([128, 2, 512], f32r, tag="r1s")
                for s in range(2):
                    ps = psA.tile([128, 512], f32, tag="ps")
                    nc.tensor.matmul(
                        ps[:], w1r[:, s * 128:(s + 1) * 128], xts[:],
                        start=True, stop=True)
                    nc.scalar.activation(
                        r1s[:, s, :], ps[:],
                        mybir.ActivationFunctionType.Relu,
                        bias=b1s[:, s:s + 1])
                for s2 in range(2):
                    ps = psA.tile([128, 512], f32, tag="ps")
                    for k in range(2):
                        nc.tensor.matmul(
                            ps[:], w2r[:, k, s2 * 128:(s2 + 1) * 128],
                            r1s[:, k, :], start=(k == 0), stop=(k == 1))
                    nc.vector.tensor_scalar(
                        hnT[s2][:, sl], ps[:], b2s[:, s2:s2 + 1], None,
                        op0=mybir.AluOpType.add)

            def corr_chunk(mt, last=False):
                base = mt * 128
                ch = chunk_pool.tile([128, CW], bf16, tag="ch")
                for t in range(9):
                    w = 512 if t < 8 else 128
                    cs = base + t * 512
                    ps = psW.tile([128, 512], f32, tag="psw")
                    for k in range(2):
                        nc.tensor.matmul(
                            ps[:, :w], hnT[k][:, base:base + 128],
                            hnT[k][:, cs:cs + w],
                            start=(k == 0), stop=(k == 1))
                    dst = ch[:, t * 512:t * 512 + w]
                    if t % 2 == 0:
                        nc.scalar.activation(
                            dst, ps[:, :w],
                            mybir.ActivationFunctionType.Identity)
                    else:
                        nc.vector.tensor_copy(dst, ps[:, :w])
                if last:
                    # quarter DMAs so the tail drains as evictions land
                    nc.sync.dma_start(out=out[mt, :, 0:1024],
                                      in_=ch[:, 0:1024])
                    nc.gpsimd.dma_start(out=out[mt, :, 1024:2048],
                                        in_=ch[:, 1024:2048])
                    nc.sync.dma_start(out=out[mt, :, 2048:3072],
                                      in_=ch[:, 2048:3072])
                    nc.gpsimd.dma_start(out=out[mt, :, 3072:CW],
                                        in_=ch[:, 3072:CW])
                else:
                    nc.sync.dma_start(out=out[mt, :, 0:2048],
                                      in_=ch[:, 0:2048])
                    nc.gpsimd.dma_start(out=out[mt, :, 2048:CW],
                                        in_=ch[:, 2048:CW])

            for nt in range(NT):
                mlp_tile(nt)
            for mt in range(CHUNKS):
                corr_chunk(mt, last=(mt == CHUNKS - 1))
    nc.compile()
    return nc


_NC_CACHE = {}


def _get_nc():
    if "nc" not in _NC_CACHE:
        _NC_CACHE["nc"] = _build_nc()
    return _NC_CACHE["nc"]


def _prep_in_maps(x, W1, b1, W2, b2):
    x = np.asarray(x, dtype=np.float32)
    W1 = np.ascontiguousarray(np.asarray(W1, dtype=np.float32))
    W2h = np.ascontiguousarray(
        np.asarray(W2, dtype=np.float32).reshape(2, 128, H).transpose(1, 0, 2))
    b1h = np.ascontiguousarray(np.asarray(b1, dtype=np.float32).reshape(2, 128).T)
    b2h = np.ascontiguousarray(np.asarray(b2, dtype=np.float32).reshape(2, 128).T)
    in_maps = []
    for m in range(NCORES):
        xTm = np.ascontiguousarray(
            np.roll(x, -SLAB * m, axis=0).T[:, :COLS])
        in_maps.append({"xT": xTm, "W1": W1, "b1": b1h, "W2": W2h,
                        "b2": b2h})
    return in_maps


def _assemble(results, edge_index):
    buf = np.stack([np.asarray(results[m]["out"]).astype(np.float32)
                    for m in range(NCORES)])  # [8, CHUNKS, 128, CW]

    g = np.arange(N)
    diag = buf[g // SLAB, (g % SLAB) // 128, g % 128, g % 128]
    rsq = 1.0 / np.maximum(np.sqrt(np.maximum(diag, 0.0)), 1e-12)

    r = np.asarray(edge_index[0], dtype=np.int64)
    c = np.asarray(edge_index[1], dtype=np.int64)
    d = (c - r) % N
    sel = d <= DMAX
    own = np.where(sel, r, c)
    off = np.where(sel, d, N - d)
    lr = own % SLAB
    p = lr % 128
    vals = buf[own // SLAB, lr // 128, p, p + off]
    vals = vals * rsq[r] * rsq[c]

    out = np.zeros((N, N), dtype=np.float32)
    out[r, c] = vals
    return out


def kernel(x, edge_index, W1, b1, W2, b2):
    nc = _get_nc()
    in_maps = _prep_in_maps(x, W1, b1, W2, b2)
    res = run_bass_kernel_spmd(nc, in_maps, list(range(NCORES)))
    return _assemble(res.results, edge_index)
